# revision 26
# baseline (speedup 1.0000x reference)
"""FWHT kernel for Trainium2: y = FWHT(x) along last axis.

x: (8192, 4096) fp32. Sharded row-wise (data-parallel) across 8 NeuronCores.

Math: FWHT (natural/Hadamard order) along an axis of 4096 equals
multiplication by Sylvester H_4096 = H_128 (x) H_32 (Kronecker).
Per row r (viewing x[r] as a 128x32 matrix X with j = 32*j1 + j0):
  Y = H_128 @ X @ H_32,   y[r, 32*i1 + i0] = Y[i1, i0]

Raw-bass 4-engine pipeline, per 16-row tile (free dim = 16*32 = 512):
  SYNC   : DMA load  x[16 rows] as [j1=128 part, (r,j0)=512 free] (128B runs)
  TENSOR : MM-B  Z = H128^T @ X  (contract j1; H sym)  -> PSUM zz[i%2]
  VECTOR : 32x32 block transpose Z -> T (j0 to partitions) -> SBUF tt[i%4]
  TENSOR : MM-A  W = blockdiag(H32 x4)^T @ T (contract j0) -> PSUM ww[i%2]
  VECTOR : 32x32 block transpose W -> O (i1 to partitions) -> SBUF oo[i%4]
  SCALAR : DMA store O as y[16 rows] [i1=128 part, (r,i0)=512 free]

Semaphores: load_sem/store_sem (+16 per DMA), pe_sem/dve_sem (+1 per op).
pe_sem after iter i: MM-B=2i+1, MM-A=2i+2. dve_sem: t=2i+1, o=2i+2.
"""

import numpy as np

N_CORES = 8
ROWS = 8192
COLS = 4096
ROWS_PER_CORE = ROWS // N_CORES  # 1024
R_TILE = 16                      # rows per matmul tile -> free dim 512
N_ITERS = ROWS_PER_CORE // R_TILE

B_IN = 8    # xin slots (prefetch depth)
B_MID = 4   # tt slots
B_OUT = 4   # oo slots

# "f32" native (4 cyc/row, exact fp32) or "f32r" (1 cyc/row at N>=256,
# ~1.5e-4 rel err; inputs pre-rounded on GPSIMD). The kernel is DMA-bound
# (strided 128B-run access pattern), so both run at the same speed; f32
# is exact.
MM_DTYPE = "f32"


def _sylvester(n: int) -> np.ndarray:
    H = np.array([[1.0]], dtype=np.float32)
    while H.shape[0] < n:
        H = np.block([[H, H], [H, -H]])
    return H.astype(np.float32)


def _h_weights() -> np.ndarray:
    h1 = _sylvester(128)
    h2 = np.kron(np.eye(4, dtype=np.float32), _sylvester(32)).astype(np.float32)
    return np.ascontiguousarray(np.concatenate([h1, h2], axis=1))


def _build_nc(n_iters: int = N_ITERS):
    import concourse.bass as bass
    import concourse.mybir as mybir

    f32 = mybir.dt.float32
    f32r = mybir.dt.float32r
    mm_dt = f32r if MM_DTYPE == "f32r" else f32

    # detect_race_conditions=False: the sim's sem-race check requires the
    # issuing engine to re-observe a semaphore between increments.  Our waits
    # use sum semantics (each DMA adds exactly +16, split 1 per SDMA engine,
    # per-engine FIFO), so >= 16*k implies the first k DMAs completed.
    nc = bass.Bass(detect_race_conditions=False)
    rows_total = n_iters * R_TILE
    x = nc.declare_dram_parameter("x", [rows_total, COLS], f32, isOutput=False)
    # h[:, 0:128] = H128, h[:, 128:256] = blockdiag(H32 x 4)
    h = nc.declare_dram_parameter("h", [128, 256], f32, isOutput=False)
    y = nc.declare_dram_parameter("y", [rows_total, COLS], f32, isOutput=True)

    use_f32r = MM_DTYPE == "f32r"
    with (
        nc.sbuf_tensor("ht", [128, 256], f32) as ht,
        nc.sbuf_tensor("htr", [128, 256], mm_dt) as htr,
        nc.sbuf_tensor("xin", [128, B_IN * 512], f32) as xin,
        nc.sbuf_tensor("xr", [128, B_IN * 512], mm_dt) as xr,
        nc.sbuf_tensor("tt", [128, B_MID * 512], f32) as tt,
        nc.sbuf_tensor("tr", [128, B_MID * 512], mm_dt) as tr,
        nc.sbuf_tensor("oo", [128, B_OUT * 512], f32) as oo,
        nc.psum_tensor("zz", [128, 2 * 512], f32) as zz,
        nc.psum_tensor("ww", [128, 2 * 512], f32) as ww,
        nc.semaphore("load_sem") as load_sem,
        nc.semaphore("store_sem") as store_sem,
        nc.semaphore("pe_sem") as pe_sem,
        nc.semaphore("dve_sem") as dve_sem,
        nc.semaphore("pool_sem") as pool_sem,
        nc.Block() as block,
    ):
        def slot(buf, i, n):
            return buf[:, (i % n) * 512:(i % n + 1) * 512]

        def xslot(i):
            return slot(xin, i, B_IN)

        def tslot(i):
            return slot(tt, i, B_MID)

        def oslot(i):
            return slot(oo, i, B_OUT)

        def zslot(i):
            return slot(zz, i, 2)

        def wslot(i):
            return slot(ww, i, 2)

        @block.sync
        def _(sync):
            sync.dma_start(ht[:], h[:]).then_inc(load_sem, 16)
            for i in range(n_iters):
                if i >= B_IN:
                    # xin slot reader must be done: MM-B(i-B_IN) (f32) or
                    # GPSIMD rounding copy (f32r)
                    if use_f32r:
                        sync.wait_ge(pool_sem, 2 * (i - B_IN) + 2)
                    else:
                        sync.wait_ge(pe_sem, 2 * (i - B_IN) + 1)
                rows = x[i * R_TILE:(i + 1) * R_TILE, :]
                sync.dma_start(
                    xslot(i).rearrange("p (r j0) -> p r j0", j0=32),
                    rows.rearrange("r (j1 j0) -> j1 r j0", j0=32),
                ).then_inc(load_sem, 16)

        if use_f32r:
            @block.gpsimd
            def _(gpsimd):
                # round weights once: pool_sem -> 1
                gpsimd.wait_ge(load_sem, 16)
                gpsimd.tensor_copy(htr[:], ht[:]).then_inc(pool_sem)
                for i in range(n_iters):
                    # round xin(i) -> xr(i): pool_sem -> 2i+2
                    gpsimd.wait_ge(load_sem, 16 * (i + 2))
                    if i >= B_IN:
                        # xr slot reader (MM-B of iter i-B_IN) must be done
                        gpsimd.wait_ge(pe_sem, 2 * (i - B_IN) + 1)
                    gpsimd.tensor_copy(
                        slot(xr, i, B_IN), xslot(i)
                    ).then_inc(pool_sem)
                    # round t(i) -> tr(i): pool_sem -> 2i+3
                    gpsimd.wait_ge(dve_sem, 2 * i + 1)
                    gpsimd.tensor_copy(
                        slot(tr, i, B_MID), tslot(i)
                    ).then_inc(pool_sem)

        @block.scalar
        def _(scalar):
            for i in range(n_iters):
                scalar.wait_ge(dve_sem, 2 * i + 2)  # o(i) ready
                yrows = y[i * R_TILE:(i + 1) * R_TILE, :]
                scalar.dma_start(
                    yrows.rearrange("r (i1 i0) -> i1 r i0", i0=32),
                    oslot(i).rearrange("p (r i0) -> p r i0", i0=32),
                ).then_inc(store_sem, 16)

        @block.tensor
        def _(tensor):
            for i in range(n_iters):
                if use_f32r:
                    tensor.wait_ge(pool_sem, 2 * i + 2)  # htr + xr(i)
                    rhs_b = slot(xr, i, B_IN)
                    lhs_b = htr[:, 0:128]
                else:
                    tensor.wait_ge(load_sem, 16 * (i + 2))  # h + xin(0..i)
                    rhs_b = xslot(i).bitcast(mm_dt)
                    lhs_b = ht[:, 0:128].bitcast(mm_dt)
                tensor.matmul(
                    out=zslot(i), lhsT=lhs_b, rhs=rhs_b, start=True, stop=True
                ).then_inc(pe_sem)  # -> 2i+1
                if use_f32r:
                    tensor.wait_ge(pool_sem, 2 * i + 3)  # tr(i) ready
                    rhs_a = slot(tr, i, B_MID)
                    lhs_a = htr[:, 128:256]
                else:
                    tensor.wait_ge(dve_sem, 2 * i + 1)  # t(i) ready
                    rhs_a = tslot(i).bitcast(mm_dt)
                    lhs_a = ht[:, 128:256].bitcast(mm_dt)
                tensor.matmul(
                    out=wslot(i), lhsT=lhs_a, rhs=rhs_a, start=True, stop=True
                ).then_inc(pe_sem)  # -> 2i+2

        @block.vector
        def _(vector):
            for i in range(n_iters):
                vector.wait_ge(pe_sem, 2 * i + 1)  # z(i) done
                vector.transpose(tslot(i), zslot(i)).then_inc(dve_sem)
                if i >= B_OUT:
                    # oo slot reader (store of iter i-B_OUT) must be done
                    vector.wait_ge(store_sem, 16 * (i - B_OUT + 1))
                vector.wait_ge(pe_sem, 2 * i + 2)  # w(i) done
                vector.transpose(oslot(i), wslot(i)).then_inc(dve_sem)

    return nc


_CACHE = {}


def kernel(x: np.ndarray) -> np.ndarray:
    from concourse.bass_utils import run_bass_kernel_spmd

    assert x.shape == (ROWS, COLS) and x.dtype == np.float32

    if "nc" not in _CACHE:
        _CACHE["nc"] = _build_nc()
    nc = _CACHE["nc"]

    h = _h_weights()

    core_ids = list(range(N_CORES))
    in_maps = [
        {
            "x": np.ascontiguousarray(x[i * ROWS_PER_CORE:(i + 1) * ROWS_PER_CORE]),
            "h": h,
        }
        for i in core_ids
    ]
    res = run_bass_kernel_spmd(nc, in_maps, core_ids)
    out = np.empty((ROWS, COLS), dtype=np.float32)
    for i in core_ids:
        out[i * ROWS_PER_CORE:(i + 1) * ROWS_PER_CORE] = res.results[i]["y"]
    return out
